# revision 8
# baseline (speedup 1.0000x reference)
"""MoE expert-network kernel for 8 Trainium2 NeuronCores.

Strategy: expert parallelism (E == n_cores == 8). The host dispatches each
token to its expert's core (an all-to-all in numpy), folds the inference-mode
BatchNorm into the expert weights/bias, and each core runs one dense
[cap, 512] @ [512, 512] GEMM fused with bias + SiLU via the activation engine.

All device tensors are laid out host-side as the exact SBUF tile images
(128-partition-major, block-contiguous per token tile) so every DMA is a
plain 2D contiguous copy with multi-KB lines.

Per-core device program (identical on all cores, SPMD):
  inputs : xs [128, KC*cap]  fp16 - token tiles, partition-major blocks
           ws [128, KC*HID]  fp16 - BN-folded weight tile image
           bs [128, MC]      fp32 - BN-folded bias tile image
  output : os [128, MC*cap]  fp16 - silu(x @ W + b), block per token tile

Pipeline (v3): everything fits in SBUF at once (~72KB/partition of 208KB),
so all load triggers are issued up front with no pool recycling. The x
tiles ride the sync HWDGE ring; the weights (split per contraction chunk)
and bias ride the scalar HWDGE ring so both rings' first transfers start in
parallel and the first matmul is gated only by (first x tile | first W
chunk) ~ 256KB. The first token tile runs its contraction outer-loop so
each W chunk is consumed as it lands. Output stores ride the sync ring
(idle once the loads have triggered); the final 128-token tile stores
per-m-chunk to shorten the last SILU->store->semaphore chain.
"""

import sys

for _p in ("/opt/trn_rl_repo",):
    if _p not in sys.path:
        sys.path.append(_p)

import numpy as np

import concourse.bass as bass
import concourse.mybir as mybir
import concourse.tile as tile
from concourse import bacc
from concourse.bass_utils import run_bass_kernel_spmd

B = 32768
IN = 512
HID = 512
E = 8
NCORES = 8
EPS = 1e-5
P = 128  # SBUF partitions
NT = 512  # matmul moving-dim chunk (one fp32 PSUM bank)

KC = IN // P  # contraction chunks
MC = HID // P  # output-feature chunks

NQ = 4  # hardware queues per dynamic DMA ring
STRIP_CONST_MEMSETS = True


def plan_sizes(cap: int) -> list:
    """Token-tile sizes: small first tile (fast pipeline fill), 1024-wide
    tiles in the middle, small last tile (short final ACT->store tail).
    Every NT-chunk of every tile is kept >= 256 columns wide: narrower
    matmuls are LDWEIGHTS-bound on the PE (128-wide runs at half rate)."""
    if cap <= 1024:
        return [cap]
    first, last = 256, 256
    body = cap - first - last
    n1024, left = divmod(body, 1024)
    if left % 512 == 128:
        # a `left` tile would end in a 128-wide chunk; fold the odd 128
        # into the first tile instead (384 is a single >=256 chunk)
        first += 128
        left -= 128
    return [first] + [1024] * n1024 + ([left] if left else []) + [last]


def build_bass(cap: int, act: str = "silu") -> bass.Bass:
    nc = bacc.Bacc(
        "TRN2",
        target_bir_lowering=False,
        debug=False,
        enable_asserts=False,
        num_devices=NCORES,
    )
    if NQ != 16:
        nc.m.queues = [
            mybir.DMAQueue(
                type=q.type,
                name=q.name,
                blocks=list(q.blocks),
                engine=q.engine,
                location_alt=q.location_alt,
                is_HWDGE=q.is_HWDGE,
                num_queues=NQ,
                num_semaphores=q.num_semaphores,
                semaphores=list(q.semaphores),
            )
            for q in nc.m.queues
        ]
    f32 = mybir.dt.float32
    f16 = mybir.dt.float16

    xs = nc.dram_tensor("xs", [P, KC * cap], f16, kind="ExternalInput").ap()
    ws = nc.dram_tensor("ws", [P, KC * HID], f16, kind="ExternalInput").ap()
    bs = nc.dram_tensor("bs", [P, MC], f32, kind="ExternalInput").ap()
    os_ = nc.dram_tensor("os", [P, MC * cap], f16, kind="ExternalOutput").ap()

    tiles = []
    n0 = 0
    for s in plan_sizes(cap):
        tiles.append((n0, s))
        n0 += s

    with tile.TileContext(nc) as tc:
        with (
            tc.tile_pool(name="wpool", bufs=KC + 1) as wpool,
            tc.tile_pool(name="xpool", bufs=len(tiles)) as xpool,
            tc.tile_pool(name="opool", bufs=len(tiles)) as opool,
            tc.tile_pool(name="pp", bufs=8, space="PSUM") as pp,
        ):
            # Bias rides the scalar ring: tiny (2KB) and needed only by the
            # first SILU, so cross-ring bandwidth arbitration can't hurt it.
            bt = wpool.tile([P, MC], f32, tag="bt", name="bt")
            nc.scalar.dma_start(out=bt, in_=bs)

            # Sync ring carries weights AND token tiles in one FIFO (a
            # separate ring's weight stream gets starved by the x flood —
            # cross-ring arbitration is not fair). Order: W chunk k=0, the
            # first token tile (first matmul gated by just these ~380KB),
            # remaining W chunks, then the rest of the token tiles.
            wts = []
            wt0 = wpool.tile([P, HID], f16, tag="wt", name="wt0")
            nc.sync.dma_start(out=wt0, in_=ws[:, :HID])
            wts.append(wt0)

            xts = []
            n0_f, nt_f = tiles[0]
            xt = xpool.tile([P, KC, nt_f], f16, tag="xt", name="xt0")
            nc.sync.dma_start(out=xt, in_=xs[:, : KC * nt_f])
            xts.append(xt)

            for k in range(1, KC):
                wtk = wpool.tile([P, HID], f16, tag="wt", name=f"wt{k}")
                nc.sync.dma_start(out=wtk, in_=ws[:, k * HID : (k + 1) * HID])
                wts.append(wtk)

            for t, (n0, nt) in enumerate(tiles[1:], start=1):
                xt = xpool.tile([P, KC, nt], f16, tag="xt", name=f"xt{t}")
                nc.sync.dma_start(out=xt, in_=xs[:, KC * n0 : KC * (n0 + nt)])
                xts.append(xt)

            for t, (n0, nt) in enumerate(tiles):
                xt = xts[t]
                ot = opool.tile([P, MC, nt], f16, tag="ot", name=f"ot{t}")
                for off in range(0, nt, NT):
                    ns = min(NT, nt - off)
                    for m in range(MC):
                        ps = pp.tile([P, ns], f32, tag="ps", name="ps")
                        for k in range(KC):
                            nc.tensor.matmul(
                                ps,
                                lhsT=wts[k][:, m * P : (m + 1) * P],
                                rhs=xt[:, k, off : off + ns],
                                start=(k == 0),
                                stop=(k == KC - 1),
                            )
                        osl = ot[:, m, off : off + ns]
                        if act == "silu":
                            nc.scalar.activation(
                                osl,
                                ps,
                                mybir.ActivationFunctionType.Silu,
                                bias=bt[:, m : m + 1],
                            )
                        else:
                            # CoreSim has no Silu: Identity+Sigmoid+mul
                            yt = opool.tile([P, ns], f32, tag="yt", name="yt")
                            nc.scalar.activation(
                                yt,
                                ps,
                                mybir.ActivationFunctionType.Identity,
                                bias=bt[:, m : m + 1],
                            )
                            st = opool.tile([P, ns], f32, tag="st", name="st")
                            nc.scalar.activation(
                                st,
                                ps,
                                mybir.ActivationFunctionType.Sigmoid,
                                bias=bt[:, m : m + 1],
                            )
                            nc.vector.tensor_mul(osl, yt, st)
                # Stores ride the sync HWDGE ring, which is idle once the
                # token-tile loads have triggered. The final (128-token)
                # tile stores per m-chunk so the last SILU->store chain is
                # as short as possible.
                if t == len(tiles) - 1:
                    for m in range(MC):
                        nc.sync.dma_start(
                            out=os_[:, MC * n0 + m * nt : MC * n0 + (m + 1) * nt],
                            in_=ot[:, m, :],
                        )
                else:
                    nc.sync.dma_start(out=os_[:, MC * n0 : MC * (n0 + nt)], in_=ot)

    if STRIP_CONST_MEMSETS:
        blk = nc.main_func.blocks[0]
        drop = [
            i
            for i in blk.instructions
            if isinstance(i, mybir.InstMemset)
            and any(
                str(getattr(o, "memref", "")).startswith("const-") for o in i.outs
            )
        ]
        for i in drop:
            blk.instructions.remove(i)

    nc.compile()
    return nc


def prepare(inputs: dict) -> tuple:
    x = np.ascontiguousarray(np.asarray(inputs["x"], dtype=np.float32))
    idx = np.asarray(inputs["expert_indices"]).astype(np.int64)
    ew = np.asarray(inputs["expert_weights"], dtype=np.float32)
    eb = np.asarray(inputs["expert_biases"], dtype=np.float32)
    gw = np.asarray(inputs["bn_weights"], dtype=np.float32)
    gb = np.asarray(inputs["bn_biases"], dtype=np.float32)
    rm = np.asarray(inputs["running_mean"], dtype=np.float32)
    rv = np.asarray(inputs["running_var"], dtype=np.float32)

    # Fold inference BN into the expert weight/bias:
    #   y = (x @ W + eb - rm) * gw/sqrt(rv+eps) + gb = x @ (W*s) + (eb-rm)*s + gb
    s = gw / np.sqrt(rv + EPS)
    wf = ew * s[:, None, :]
    bf = (eb - rm) * s + gb

    perms = [np.nonzero(idx == e)[0] for e in range(E)]
    counts = [len(p) for p in perms]
    cap = max(512, -(-max(counts) // P) * P)
    tiles = []
    n0 = 0
    for t in plan_sizes(cap):
        tiles.append((n0, t))
        n0 += t

    in_maps = []
    for e in range(E):
        xT = np.zeros((IN, cap), dtype=np.float16)
        if counts[e]:
            xT[:, : counts[e]] = x[perms[e]].T.astype(np.float16)
        xv = xT.reshape(KC, P, cap)
        xs = np.empty((P, KC * cap), dtype=np.float16)
        for n0, nt in tiles:
            xs[:, KC * n0 : KC * (n0 + nt)] = (
                xv[:, :, n0 : n0 + nt].transpose(1, 0, 2).reshape(P, KC * nt)
            )
        ws = (
            wf[e]
            .astype(np.float16)
            .reshape(KC, P, HID)
            .transpose(1, 0, 2)
            .reshape(P, KC * HID)
        )
        bs = np.ascontiguousarray(bf[e].reshape(MC, P).T)
        in_maps.append({"xs": xs, "ws": np.ascontiguousarray(ws), "bs": bs})
    return cap, tiles, perms, counts, in_maps


def combine(results: list, cap, tiles, perms, counts) -> np.ndarray:
    out = np.empty((B, HID), dtype=np.float32)
    for e in range(E):
        if not counts[e]:
            continue
        ob = results[e]["os"]
        oT = np.empty((HID, cap), dtype=np.float32)
        for n0, nt in tiles:
            oT[:, n0 : n0 + nt] = (
                ob[:, MC * n0 : MC * (n0 + nt)]
                .reshape(P, MC, nt)
                .transpose(1, 0, 2)
                .reshape(HID, nt)
            )
        out[perms[e]] = oT[:, : counts[e]].T
    return out


def kernel(**inputs) -> np.ndarray:
    cap, tiles, perms, counts, in_maps = prepare(inputs)
    nc = build_bass(cap)
    res = run_bass_kernel_spmd(nc, in_maps, core_ids=list(range(NCORES)))
    return combine(res.results, cap, tiles, perms, counts)


# revision 9
# speedup vs baseline: 2.0272x; 2.0272x over previous
"""MoE expert-network kernel for 8 Trainium2 NeuronCores.

Strategy: expert parallelism (E == n_cores == 8). The host dispatches each
token to its expert's core (an all-to-all in numpy), folds the inference-mode
BatchNorm into the expert weights/bias, and each core runs one dense
[cap, 512] @ [512, 512] GEMM fused with bias + SiLU via the activation engine.

All device tensors are laid out host-side as the exact SBUF tile images
(128-partition-major, block-contiguous per token tile) so every DMA is a
plain 2D contiguous copy with multi-KB lines.

Per-core device program (identical on all cores, SPMD):
  inputs : xs [128, KC*cap]  fp16 - token tiles, partition-major blocks
           ws [128, KC*HID]  fp16 - BN-folded weight tile image
           bs [128, MC]      fp32 - BN-folded bias tile image
  output : os [128, MC*cap]  fp16 - silu(x @ W + b), block per token tile

Pipeline (v3): everything fits in SBUF at once (~72KB/partition of 208KB),
so all load triggers are issued up front with no pool recycling. The x
tiles ride the sync HWDGE ring; the weights (split per contraction chunk)
and bias ride the scalar HWDGE ring so both rings' first transfers start in
parallel and the first matmul is gated only by (first x tile | first W
chunk) ~ 256KB. The first token tile runs its contraction outer-loop so
each W chunk is consumed as it lands. Output stores ride the sync ring
(idle once the loads have triggered); the final 128-token tile stores
per-m-chunk to shorten the last SILU->store->semaphore chain.
"""

import sys

for _p in ("/opt/trn_rl_repo",):
    if _p not in sys.path:
        sys.path.append(_p)

import numpy as np

import concourse.bass as bass
import concourse.mybir as mybir
import concourse.tile as tile
from concourse import bacc
from concourse.bass_utils import run_bass_kernel_spmd

B = 32768
IN = 512
HID = 512
E = 8
NCORES = 8
EPS = 1e-5
P = 128  # SBUF partitions
NT = 512  # matmul moving-dim chunk (one fp32 PSUM bank)

KC = IN // P  # contraction chunks
MC = HID // P  # output-feature chunks

NQ = 16  # hardware queues per dynamic DMA ring
STRIP_CONST_MEMSETS = True


def plan_sizes(cap: int) -> list:
    """Token-tile sizes: small first tile (fast pipeline fill), 1024-wide
    tiles in the middle, small last tile (short final ACT->store tail).
    Every NT-chunk of every tile is kept >= 256 columns wide: narrower
    matmuls are LDWEIGHTS-bound on the PE (128-wide runs at half rate)."""
    if cap <= 1024:
        return [cap]
    first, last = 256, 256
    body = cap - first - last
    n1024, left = divmod(body, 1024)
    if left % 512 == 128:
        # a `left` tile would end in a 128-wide chunk; fold the odd 128
        # into the first tile instead (384 is a single >=256 chunk)
        first += 128
        left -= 128
    return [first] + [1024] * n1024 + ([left] if left else []) + [last]


def build_bass(cap: int, act: str = "silu") -> bass.Bass:
    nc = bacc.Bacc(
        "TRN2",
        target_bir_lowering=False,
        debug=False,
        enable_asserts=False,
        num_devices=NCORES,
    )
    if NQ != 16:
        nc.m.queues = [
            mybir.DMAQueue(
                type=q.type,
                name=q.name,
                blocks=list(q.blocks),
                engine=q.engine,
                location_alt=q.location_alt,
                is_HWDGE=q.is_HWDGE,
                num_queues=NQ,
                num_semaphores=q.num_semaphores,
                semaphores=list(q.semaphores),
            )
            for q in nc.m.queues
        ]
    f32 = mybir.dt.float32
    f16 = mybir.dt.float16

    xs = nc.dram_tensor("xs", [P, KC * cap], f16, kind="ExternalInput").ap()
    ws = nc.dram_tensor("ws", [P, KC * HID], f16, kind="ExternalInput").ap()
    bs = nc.dram_tensor("bs", [P, MC], f32, kind="ExternalInput").ap()
    os_ = nc.dram_tensor("os", [P, MC * cap], f16, kind="ExternalOutput").ap()

    tiles = []
    n0 = 0
    for s in plan_sizes(cap):
        tiles.append((n0, s))
        n0 += s

    with tile.TileContext(nc) as tc:
        with (
            tc.tile_pool(name="wpool", bufs=KC + 1) as wpool,
            tc.tile_pool(name="xpool", bufs=len(tiles)) as xpool,
            tc.tile_pool(name="opool", bufs=len(tiles)) as opool,
            tc.tile_pool(name="pp", bufs=8, space="PSUM") as pp,
        ):
            # Bias rides the scalar ring: tiny (2KB) and needed only by the
            # first SILU, so cross-ring bandwidth arbitration can't hurt it.
            bt = wpool.tile([P, MC], f32, tag="bt", name="bt")
            nc.scalar.dma_start(out=bt, in_=bs)

            # Sync ring carries weights AND token tiles in one FIFO (a
            # separate ring's weight stream gets starved by the x flood —
            # cross-ring arbitration is not fair). Order: W chunk k=0, the
            # first token tile (first matmul gated by just these ~380KB),
            # remaining W chunks, then the rest of the token tiles.
            wts = []
            wt0 = wpool.tile([P, HID], f16, tag="wt", name="wt0")
            nc.sync.dma_start(out=wt0, in_=ws[:, :HID])
            wts.append(wt0)

            xts = []
            n0_f, nt_f = tiles[0]
            xt = xpool.tile([P, KC, nt_f], f16, tag="xt", name="xt0")
            nc.sync.dma_start(out=xt, in_=xs[:, : KC * nt_f])
            xts.append(xt)

            for k in range(1, KC):
                wtk = wpool.tile([P, HID], f16, tag="wt", name=f"wt{k}")
                nc.sync.dma_start(out=wtk, in_=ws[:, k * HID : (k + 1) * HID])
                wts.append(wtk)

            for t, (n0, nt) in enumerate(tiles[1:], start=1):
                xt = xpool.tile([P, KC, nt], f16, tag="xt", name=f"xt{t}")
                nc.sync.dma_start(out=xt, in_=xs[:, KC * n0 : KC * (n0 + nt)])
                xts.append(xt)

            for t, (n0, nt) in enumerate(tiles):
                xt = xts[t]
                ot = opool.tile([P, MC, nt], f16, tag="ot", name=f"ot{t}")
                for off in range(0, nt, NT):
                    ns = min(NT, nt - off)
                    for m in range(MC):
                        ps = pp.tile([P, ns], f32, tag="ps", name="ps")
                        for k in range(KC):
                            nc.tensor.matmul(
                                ps,
                                lhsT=wts[k][:, m * P : (m + 1) * P],
                                rhs=xt[:, k, off : off + ns],
                                start=(k == 0),
                                stop=(k == KC - 1),
                            )
                        osl = ot[:, m, off : off + ns]
                        if act == "silu":
                            nc.scalar.activation(
                                osl,
                                ps,
                                mybir.ActivationFunctionType.Silu,
                                bias=bt[:, m : m + 1],
                            )
                        else:
                            # CoreSim has no Silu: Identity+Sigmoid+mul
                            yt = opool.tile([P, ns], f32, tag="yt", name="yt")
                            nc.scalar.activation(
                                yt,
                                ps,
                                mybir.ActivationFunctionType.Identity,
                                bias=bt[:, m : m + 1],
                            )
                            st = opool.tile([P, ns], f32, tag="st", name="st")
                            nc.scalar.activation(
                                st,
                                ps,
                                mybir.ActivationFunctionType.Sigmoid,
                                bias=bt[:, m : m + 1],
                            )
                            nc.vector.tensor_mul(osl, yt, st)
                # Stores ride the sync HWDGE ring, which is idle once the
                # token-tile loads have triggered. The final (128-token)
                # tile stores per m-chunk so the last SILU->store chain is
                # as short as possible.
                if t == len(tiles) - 1:
                    for m in range(MC):
                        nc.sync.dma_start(
                            out=os_[:, MC * n0 + m * nt : MC * n0 + (m + 1) * nt],
                            in_=ot[:, m, :],
                        )
                else:
                    nc.sync.dma_start(out=os_[:, MC * n0 : MC * (n0 + nt)], in_=ot)

    if STRIP_CONST_MEMSETS:
        blk = nc.main_func.blocks[0]
        drop = [
            i
            for i in blk.instructions
            if isinstance(i, mybir.InstMemset)
            and any(
                str(getattr(o, "memref", "")).startswith("const-") for o in i.outs
            )
        ]
        for i in drop:
            blk.instructions.remove(i)

    nc.compile()
    return nc


def prepare(inputs: dict) -> tuple:
    x = np.ascontiguousarray(np.asarray(inputs["x"], dtype=np.float32))
    idx = np.asarray(inputs["expert_indices"]).astype(np.int64)
    ew = np.asarray(inputs["expert_weights"], dtype=np.float32)
    eb = np.asarray(inputs["expert_biases"], dtype=np.float32)
    gw = np.asarray(inputs["bn_weights"], dtype=np.float32)
    gb = np.asarray(inputs["bn_biases"], dtype=np.float32)
    rm = np.asarray(inputs["running_mean"], dtype=np.float32)
    rv = np.asarray(inputs["running_var"], dtype=np.float32)

    # Fold inference BN into the expert weight/bias:
    #   y = (x @ W + eb - rm) * gw/sqrt(rv+eps) + gb = x @ (W*s) + (eb-rm)*s + gb
    s = gw / np.sqrt(rv + EPS)
    wf = ew * s[:, None, :]
    bf = (eb - rm) * s + gb

    perms = [np.nonzero(idx == e)[0] for e in range(E)]
    counts = [len(p) for p in perms]
    cap = max(512, -(-max(counts) // P) * P)
    tiles = []
    n0 = 0
    for t in plan_sizes(cap):
        tiles.append((n0, t))
        n0 += t

    in_maps = []
    for e in range(E):
        xT = np.zeros((IN, cap), dtype=np.float16)
        if counts[e]:
            xT[:, : counts[e]] = x[perms[e]].T.astype(np.float16)
        xv = xT.reshape(KC, P, cap)
        xs = np.empty((P, KC * cap), dtype=np.float16)
        for n0, nt in tiles:
            xs[:, KC * n0 : KC * (n0 + nt)] = (
                xv[:, :, n0 : n0 + nt].transpose(1, 0, 2).reshape(P, KC * nt)
            )
        ws = (
            wf[e]
            .astype(np.float16)
            .reshape(KC, P, HID)
            .transpose(1, 0, 2)
            .reshape(P, KC * HID)
        )
        bs = np.ascontiguousarray(bf[e].reshape(MC, P).T)
        in_maps.append({"xs": xs, "ws": np.ascontiguousarray(ws), "bs": bs})
    return cap, tiles, perms, counts, in_maps


def combine(results: list, cap, tiles, perms, counts) -> np.ndarray:
    out = np.empty((B, HID), dtype=np.float32)
    for e in range(E):
        if not counts[e]:
            continue
        ob = results[e]["os"]
        oT = np.empty((HID, cap), dtype=np.float32)
        for n0, nt in tiles:
            oT[:, n0 : n0 + nt] = (
                ob[:, MC * n0 : MC * (n0 + nt)]
                .reshape(P, MC, nt)
                .transpose(1, 0, 2)
                .reshape(HID, nt)
            )
        out[perms[e]] = oT[:, : counts[e]].T
    return out


def kernel(**inputs) -> np.ndarray:
    cap, tiles, perms, counts, in_maps = prepare(inputs)
    nc = build_bass(cap)
    res = run_bass_kernel_spmd(nc, in_maps, core_ids=list(range(NCORES)))
    return combine(res.results, cap, tiles, perms, counts)
